# revision 17
# baseline (speedup 1.0000x reference)
"""Trainium2 Bass kernel for AdvancedMoEMixtureLoRA - V9.

Reference computation (per token t of N = 4*2048 = 8192, D = 4096):
    z        = x @ A_w.T                       [N, 16]
    M        = 8 * (x @ M_w.T + M_b)           [N, 256] -> [N, 16, 16]
    z_mixed  = M @ z  (per token matvec)       [N, 16]
    out      = 128 * z_mixed @ B_w.T           [N, 4096]

Strategy: pure data parallel over tokens (1024 tokens per core, weights
replicated, no collectives).  Host-side prep (free, not on HW critical
path): transpose x to d-major per 128-token slab, cast everything to
bf16, fuse A_w/M_w into one [4096, 272] weight, fold all scalar factors
into the weights.

Bias trick: with MB = 8*M_b.reshape(16,16),
    out = (128 B_w) @ (M_hat z) + (128 B_w MB) @ z,   M_hat = x@(8 M_w).T
so the M_b bias folds into a K=32 B matmul (stationary = [z_mixedT; zT],
weights = [(128 B_w).T ; (128 B_w MB).T]) at zero extra PE time.

V9 vs V7:
  - transpose moved off the PE: DVE 32x32 StreamTranspose (4 blocks)
    builds yT in SBUF; drops the identity input, the PE transpose
    (8x275ns) and one PSUM bank.
  - the DVE mul reads z straight from the AM PSUM (no serial ACT copy
    in front of it).
  - B outputs go to 3 double-bank PSUM tiles (2 matmuls each), drained
    by 1024-wide evacuations: ACT takes the first two, DVE (after the
    mix chain) the last two.  PSUM: 2 AM + 6 B banks.
  - stores are one 1MB transfer per chunk on the sync FIFO behind all
    loads (fewer end-of-run semaphore events, which serialize into the
    final barrier at ~115ns each on the waiting sequencers).
"""

import sys

if "/opt/trn_rl_repo" not in sys.path:
    sys.path.insert(0, "/opt/trn_rl_repo")

import ml_dtypes
import numpy as np

import concourse.bass as bass
import concourse.tile as tile
from concourse import bacc, mybir
from concourse.bass_utils import run_bass_kernel_spmd

N_CORES = 8
B, S, D = 4, 2048, 4096
N_TOK = B * S                # 8192
TPC = N_TOK // N_CORES       # tokens per core = 1024
CHUNK = 128                  # tokens per PSUM chunk
NCHUNK = TPC // CHUNK        # 8
RH = 16                      # lora rank*heads
MDIM = RH * RH               # 256
WCOLS = MDIM + RH            # 272 fused output cols (M | z)
KD = D // 128                # 32 d-chunks
OUT_D = 4096

BF = mybir.dt.bfloat16
F32 = mybir.dt.float32
NPBF = ml_dtypes.bfloat16


def build_nc():
    nc = bacc.Bacc("TRN2", target_bir_lowering=False, debug=False)
    # host-swizzled x: xsw[p, c*(KD*CHUNK) + k*CHUNK + t] = x[c*CHUNK + t, k*128 + p]
    xsw = nc.dram_tensor("xsw", [128, NCHUNK * KD * CHUNK], BF, kind="ExternalInput").ap()
    # host-swizzled W: wsw[p, k*WCOLS + m] = W.T[k*128 + p, m]
    wsw = nc.dram_tensor("wsw", [128, KD * WCOLS], BF, kind="ExternalInput").ap()
    # stacked B weights: rows 0-15 = (128 B_w).T, rows 16-31 = (128 B_w MB).T
    bT = nc.dram_tensor("bT", [2 * RH, OUT_D], BF, kind="ExternalInput").ap()
    out = nc.dram_tensor("out", [TPC, OUT_D], BF, kind="ExternalOutput").ap()

    SLAB = KD * CHUNK  # 4096 cols per token-slab

    with tile.TileContext(nc) as tc:
        with (
            tc.tile_pool(name="xpool", bufs=8) as xpool,
            tc.tile_pool(name="wpool", bufs=1) as wpool,
            tc.tile_pool(name="cpool", bufs=1) as cpool,
            tc.tile_pool(name="mix", bufs=3) as mixpool,
            tc.tile_pool(name="osb", bufs=8) as opool,
            tc.tile_pool(name="am", bufs=2, space="PSUM") as ampool,
            tc.tile_pool(name="bp", bufs=3, space="PSUM") as bpool,
        ):
            # All PE-critical loads share the SP HWDGE queue (strict FIFO)
            # in explicit dependency order: W pieces front-loaded between
            # the first x slab halves.
            wsb = wpool.tile([128, KD, WCOLS], BF)
            wflat = wsb[:].rearrange("p k m -> p (k m)")
            WQ = 4 * WCOLS

            xtiles = [
                xpool.tile([128, KD, CHUNK], BF, name=f"xs{c}", tag="xs")
                for c in range(NCHUNK)
            ]
            HS = SLAB // 2

            def load_w(q):
                nc.sync.dma_start(wflat[:, q * WQ:(q + 1) * WQ], wsw[:, q * WQ:(q + 1) * WQ])

            def load_x(c, hh):
                xsflat = xtiles[c][:].rearrange("p k t -> p (k t)")
                nc.sync.dma_start(
                    xsflat[:, hh * HS:(hh + 1) * HS],
                    xsw[:, c * SLAB + hh * HS:c * SLAB + (hh + 1) * HS],
                )

            # first slab in quarters so AM(0, k<8) starts ASAP.
            # the whole x0 slab rides the (idle until ~18us) scalar HWDGE
            # queue, overlapping the W stream on sync instead of
            # serializing behind it; w0 split so AM(0,k=0) starts sooner
            QS = SLAB // 4
            x0flat = xtiles[0][:].rearrange("p k t -> p (k t)")
            nc.scalar.dma_start(x0flat[:, 0:QS], xsw[:, 0:QS])
            nc.scalar.dma_start(x0flat[:, QS:2 * QS], xsw[:, QS:2 * QS])
            WH = WQ // 2
            nc.sync.dma_start(wflat[:, 0:WH], wsw[:, 0:WH])
            nc.sync.dma_start(wflat[:, WH:WQ], wsw[:, WH:WQ])
            load_w(1); load_w(2)
            nc.sync.dma_start(x0flat[:, 2 * QS:3 * QS], xsw[:, 2 * QS:3 * QS])
            load_w(3)
            nc.sync.dma_start(x0flat[:, 3 * QS:4 * QS], xsw[:, 3 * QS:4 * QS])
            load_w(4); load_w(5); load_w(6); load_w(7)
            load_x(1, 0); load_x(1, 1)
            # x2 stays half-grain: it is the transition chunk where the
            # AM stream catches the supply edge
            load_x(2, 0); load_x(2, 1)
            for c in range(3, NCHUNK):
                xsflat = xtiles[c][:].rearrange("p k t -> p (k t)")
                nc.sync.dma_start(xsflat[:], xsw[:, c * SLAB:(c + 1) * SLAB])

            btsb = cpool.tile([2 * RH, OUT_D], BF)
            nc.gpsimd.dma_start(btsb[:], bT)

            # PE warm-up: HAM (full-clock high-activity mode) engages only
            # ~12us after the PE first goes busy; without this, the first
            # 2-3 chunks run at half clock (227ns vs 116ns per matmul).
            # Dummy matmuls on a memset tile pull the qualification window
            # forward to overlap the W/x0 load latency.
            wu_w = cpool.tile([128, 128], BF)
            nc.vector.memset(wu_w[:], 0.0)
            wu_r = opool.tile([128, 512], BF, name="wu_r", tag="osb")
            nc.gpsimd.memset(wu_r[:], 0.0)
            wu_ps = ampool.tile([128, 512], F32, name="wu", tag="am")
            for w in range(10):
                nc.tensor.matmul(
                    wu_ps[:], lhsT=wu_w[:], rhs=wu_r[:],
                    start=True, stop=True,
                )

            zts = [None] * NCHUNK    # per-chunk [32, 128] bf16 stationary
            osbs = [None] * NCHUNK   # per-chunk [128, 4096] bf16 out tiles

            def mix_tail(c, am, last=False):
                """After AM(c) PSUM completes: DVE mul (z read straight
                from PSUM), DVE reduce, ACT z copy, DVE 32x32 transposes,
                ACT bf16 downcast of the transposed stationary."""
                zc = mixpool.tile([128, 2 * RH], F32, tag="zc", name=f"zc{c}")
                # z copy (ACT): DVE cannot read both mul operands from PSUM
                nc.scalar.copy(zc[:, RH:2 * RH], am[:, MDIM:WCOLS])
                # P[p, i, j] = M[p, i, j] * z[p, j]
                p_sb = mixpool.tile([128, MDIM], BF, tag="p", name=f"pp{c}")
                nc.vector.tensor_mul(
                    p_sb[:].rearrange("p (i j) -> p i j", i=RH),
                    am[:, 0:MDIM].rearrange("p (i j) -> p i j", i=RH),
                    zc[:, RH:2 * RH].unsqueeze(1).broadcast_to([128, RH, RH]),
                )
                # z_mixed[p, i] = sum_j P[p, i, j]
                nc.vector.tensor_reduce(
                    zc[:, 0:RH], p_sb[:].rearrange("p (i j) -> p i j", i=RH),
                    axis=mybir.AxisListType.X, op=mybir.AluOpType.add,
                )
                # transpose [z_mixed | z] -> [32, 128] via 4 DVE 32x32 blocks
                zt_f = mixpool.tile([2 * RH, CHUNK], F32, tag="ztf", name=f"ztf{c}")
                for b in range(4):
                    nc.vector.transpose(
                        zt_f[:, b * 32:(b + 1) * 32],
                        zc[b * 32:(b + 1) * 32, :],
                    )
                # bf16 downcast: DVE normally (194ns right behind the
                # transposes); for the LAST chunk use Pool — the scheduler
                # otherwise hoists burst(6) evacuations above this copy on
                # DVE and the final burst slips into a clock-down window
                zt_sb = mixpool.tile([2 * RH, CHUNK], BF, tag="zt", name=f"zt{c}")
                if last:
                    nc.gpsimd.tensor_copy(zt_sb[:], zt_f[:])
                else:
                    nc.vector.tensor_copy(zt_sb[:], zt_f[:])
                zts[c] = zt_sb

            def b_burst(c, fast_tail=False):
                """8 B matmuls (K=32, N=512) into 3 double-bank PSUM
                tiles; 1024-wide evacuations ACT, ACT, DVE, DVE; stores
                of each 2048 half ride the sync FIFO behind the loads."""
                o_sb = opool.tile([128, OUT_D], BF, name=f"osb{c}", tag="osb")
                osbs[c] = o_sb
                tok = slice(c * CHUNK, (c + 1) * CHUNK)
                for pair in range(4):
                    bp = bpool.tile([128, 1024], F32, name=f"bp{c}_{pair}", tag="bp")
                    for half in range(2):
                        ob = 2 * pair + half
                        nc.tensor.matmul(
                            bp[:, half * 512:(half + 1) * 512],
                            lhsT=zts[c][:], rhs=btsb[:, ob * 512:(ob + 1) * 512],
                            start=True, stop=True,
                        )
                    osl = slice(pair * 1024, (pair + 1) * 1024)
                    if pair % 2 == 0:
                        nc.scalar.copy(o_sb[:, osl], bp[:])
                    else:
                        nc.vector.tensor_copy(o_sb[:, osl], bp[:])
                    if fast_tail:
                        # final chunk: store each 1024 as it lands, spread
                        # across the three HWDGE queues
                        q = [nc.sync, nc.scalar, nc.gpsimd, nc.scalar]
                        q[pair].dma_start(out[tok, osl], o_sb[:, osl])
                    elif pair == 3:
                        # one 1MB store per chunk: fewer end-of-run
                        # semaphore events (they serialize into the final
                        # barrier at ~115ns apiece)
                        nc.sync.dma_start(out[tok, :], o_sb[:])

            # software pipeline: per chunk the PE order is [AM(c), B(c-1)];
            # the mix chain of c is EMITTED before the burst of c-1 so the
            # DVE/ACT queues serve it first and zt(c) is ready well before
            # the PE reaches B(c)
            for c in range(NCHUNK):
                xs = xtiles[c]
                am = ampool.tile([128, WCOLS], F32, name=f"am{c}", tag="am")
                for k in range(KD):
                    nc.tensor.matmul(
                        am[:], lhsT=xs[:, k, :], rhs=wsb[:, k, :],
                        start=(k == 0), stop=(k == KD - 1),
                    )
                mix_tail(c, am, last=(c == NCHUNK - 1))
                if c >= 1:
                    b_burst(c - 1)
                if c == NCHUNK - 1:
                    # warm-keeper: the PE otherwise idles ~2us here waiting
                    # for zt(7), dropping out of high-activity mode and
                    # running the last burst at half clock.  Harmless
                    # matmuls into the retired am tile keep HAM up.
                    for w in range(16):
                        nc.tensor.matmul(
                            am[:], lhsT=xs[:, w % KD, :], rhs=wsb[:, w % KD, :],
                            start=True, stop=True,
                        )
            b_burst(NCHUNK - 1, fast_tail=True)

    nc.compile()
    return nc


_NC = None


def _get_nc():
    global _NC
    if _NC is None:
        _NC = build_nc()
    return _NC


def make_in_maps(x, A_w, B_w, M_w, M_b):
    x = np.asarray(x, dtype=np.float32)
    A_w = np.asarray(A_w, dtype=np.float32)
    B_w = np.asarray(B_w, dtype=np.float32)
    M_w = np.asarray(M_w, dtype=np.float32)
    M_b = np.asarray(M_b, dtype=np.float32)

    # fold scales: M_hat = x @ (8 M_w).T ; out = z_mixed @ (128 B_w).T + z @ (128 B_w MB).T
    W = np.concatenate([8.0 * M_w, A_w], axis=0)              # [272, 4096]
    wT_np = W.T.astype(NPBF)                                  # [4096, 272]
    # swizzle to [128, k*272 + m] so each SBUF partition line is contiguous
    wsw_np = np.ascontiguousarray(
        wT_np.reshape(KD, 128, WCOLS).transpose(1, 0, 2).reshape(128, KD * WCOLS)
    )
    MB = (8.0 * M_b).reshape(RH, RH)
    B1 = 128.0 * B_w                                          # [4096, 16]
    B2 = B1 @ MB                                              # [4096, 16]
    bT_np = np.ascontiguousarray(
        np.concatenate([B1.T, B2.T], axis=0)                  # [32, 4096]
    ).astype(NPBF)

    xf = x.reshape(N_TOK, D)
    in_maps = []
    for core in range(N_CORES):
        shard = xf[core * TPC:(core + 1) * TPC].astype(NPBF)  # [1024, 4096]
        # xsw[p, c*4096 + k*128 + t] = shard[c*128 + t, k*128 + p]
        xsw_np = np.ascontiguousarray(
            shard.reshape(NCHUNK, CHUNK, KD, 128)             # [c, t, k, p]
            .transpose(3, 0, 2, 1)                            # [p, c, k, t]
            .reshape(128, NCHUNK * KD * CHUNK)
        )
        in_maps.append({
            "xsw": xsw_np, "wsw": wsw_np, "bT": bT_np,
        })
    return in_maps


def assemble_out(results):
    outs = [np.asarray(results[i]["out"], dtype=np.float32) for i in range(N_CORES)]
    return np.concatenate(outs, axis=0).reshape(B, S, OUT_D)


def kernel(x, A_w, B_w, M_w, M_b):
    nc = _get_nc()
    in_maps = make_in_maps(x, A_w, B_w, M_w, M_b)
    res = run_bass_kernel_spmd(nc, in_maps, core_ids=list(range(N_CORES)))
    return assemble_out(res.results)


# revision 18
# speedup vs baseline: 1.0363x; 1.0363x over previous
"""Trainium2 Bass kernel for AdvancedMoEMixtureLoRA - V9.

Reference computation (per token t of N = 4*2048 = 8192, D = 4096):
    z        = x @ A_w.T                       [N, 16]
    M        = 8 * (x @ M_w.T + M_b)           [N, 256] -> [N, 16, 16]
    z_mixed  = M @ z  (per token matvec)       [N, 16]
    out      = 128 * z_mixed @ B_w.T           [N, 4096]

Strategy: pure data parallel over tokens (1024 tokens per core, weights
replicated, no collectives).  Host-side prep (free, not on HW critical
path): transpose x to d-major per 128-token slab, cast everything to
bf16, fuse A_w/M_w into one [4096, 272] weight, fold all scalar factors
into the weights.

Bias trick: with MB = 8*M_b.reshape(16,16),
    out = (128 B_w) @ (M_hat z) + (128 B_w MB) @ z,   M_hat = x@(8 M_w).T
so the M_b bias folds into a K=32 B matmul (stationary = [z_mixedT; zT],
weights = [(128 B_w).T ; (128 B_w MB).T]) at zero extra PE time.

V9 vs V7:
  - transpose moved off the PE: DVE 32x32 StreamTranspose (4 blocks)
    builds yT in SBUF; drops the identity input, the PE transpose
    (8x275ns) and one PSUM bank.
  - the DVE mul reads z straight from the AM PSUM (no serial ACT copy
    in front of it).
  - B outputs go to 3 double-bank PSUM tiles (2 matmuls each), drained
    by 1024-wide evacuations: ACT takes the first two, DVE (after the
    mix chain) the last two.  PSUM: 2 AM + 6 B banks.
  - stores are one 1MB transfer per chunk on the sync FIFO behind all
    loads (fewer end-of-run semaphore events, which serialize into the
    final barrier at ~115ns each on the waiting sequencers).
"""

import sys

if "/opt/trn_rl_repo" not in sys.path:
    sys.path.insert(0, "/opt/trn_rl_repo")

import ml_dtypes
import numpy as np

import concourse.bass as bass
import concourse.tile as tile
from concourse import bacc, mybir
from concourse.bass_utils import run_bass_kernel_spmd

N_CORES = 8
B, S, D = 4, 2048, 4096
N_TOK = B * S                # 8192
TPC = N_TOK // N_CORES       # tokens per core = 1024
CHUNK = 128                  # tokens per PSUM chunk
NCHUNK = TPC // CHUNK        # 8
RH = 16                      # lora rank*heads
MDIM = RH * RH               # 256
WCOLS = MDIM + RH            # 272 fused output cols (M | z)
KD = D // 128                # 32 d-chunks
OUT_D = 4096

BF = mybir.dt.bfloat16
F32 = mybir.dt.float32
NPBF = ml_dtypes.bfloat16


def build_nc():
    nc = bacc.Bacc("TRN2", target_bir_lowering=False, debug=False)
    # host-swizzled x: xsw[p, c*(KD*CHUNK) + k*CHUNK + t] = x[c*CHUNK + t, k*128 + p]
    xsw = nc.dram_tensor("xsw", [128, NCHUNK * KD * CHUNK], BF, kind="ExternalInput").ap()
    # host-swizzled W: wsw[p, k*WCOLS + m] = W.T[k*128 + p, m]
    wsw = nc.dram_tensor("wsw", [128, KD * WCOLS], BF, kind="ExternalInput").ap()
    # stacked B weights: rows 0-15 = (128 B_w).T, rows 16-31 = (128 B_w MB).T
    bT = nc.dram_tensor("bT", [2 * RH, OUT_D], BF, kind="ExternalInput").ap()
    out = nc.dram_tensor("out", [TPC, OUT_D], BF, kind="ExternalOutput").ap()

    SLAB = KD * CHUNK  # 4096 cols per token-slab

    with tile.TileContext(nc) as tc:
        with (
            tc.tile_pool(name="xpool", bufs=8) as xpool,
            tc.tile_pool(name="wpool", bufs=1) as wpool,
            tc.tile_pool(name="cpool", bufs=1) as cpool,
            tc.tile_pool(name="mix", bufs=3) as mixpool,
            tc.tile_pool(name="osb", bufs=8) as opool,
            tc.tile_pool(name="am", bufs=2, space="PSUM") as ampool,
            tc.tile_pool(name="bp", bufs=3, space="PSUM") as bpool,
        ):
            # All PE-critical loads share the SP HWDGE queue (strict FIFO)
            # in explicit dependency order: W pieces front-loaded between
            # the first x slab halves.
            wsb = wpool.tile([128, KD, WCOLS], BF)
            wflat = wsb[:].rearrange("p k m -> p (k m)")
            WQ = 4 * WCOLS

            xtiles = [
                xpool.tile([128, KD, CHUNK], BF, name=f"xs{c}", tag="xs")
                for c in range(NCHUNK)
            ]
            HS = SLAB // 2

            def load_w(q):
                nc.sync.dma_start(wflat[:, q * WQ:(q + 1) * WQ], wsw[:, q * WQ:(q + 1) * WQ])

            def load_x(c, hh):
                xsflat = xtiles[c][:].rearrange("p k t -> p (k t)")
                nc.sync.dma_start(
                    xsflat[:, hh * HS:(hh + 1) * HS],
                    xsw[:, c * SLAB + hh * HS:c * SLAB + (hh + 1) * HS],
                )

            # first slab in quarters so AM(0, k<8) starts ASAP.
            # the whole x0 slab rides the (idle until ~18us) scalar HWDGE
            # queue, overlapping the W stream on sync instead of
            # serializing behind it; w0 split so AM(0,k=0) starts sooner
            QS = SLAB // 4
            x0flat = xtiles[0][:].rearrange("p k t -> p (k t)")
            nc.scalar.dma_start(x0flat[:, 0:QS], xsw[:, 0:QS])
            nc.scalar.dma_start(x0flat[:, QS:2 * QS], xsw[:, QS:2 * QS])
            WH = WQ // 2
            nc.sync.dma_start(wflat[:, 0:WH], wsw[:, 0:WH])
            nc.sync.dma_start(wflat[:, WH:WQ], wsw[:, WH:WQ])
            load_w(1); load_w(2)
            nc.sync.dma_start(x0flat[:, 2 * QS:3 * QS], xsw[:, 2 * QS:3 * QS])
            load_w(3)
            nc.sync.dma_start(x0flat[:, 3 * QS:4 * QS], xsw[:, 3 * QS:4 * QS])
            load_w(4); load_w(5); load_w(6); load_w(7)
            load_x(1, 0); load_x(1, 1)
            # x2 stays half-grain: it is the transition chunk where the
            # AM stream catches the supply edge
            load_x(2, 0); load_x(2, 1)
            for c in range(3, NCHUNK):
                xsflat = xtiles[c][:].rearrange("p k t -> p (k t)")
                nc.sync.dma_start(xsflat[:], xsw[:, c * SLAB:(c + 1) * SLAB])

            btsb = cpool.tile([2 * RH, OUT_D], BF)
            nc.gpsimd.dma_start(btsb[:], bT)

            # PE warm-up: HAM (full-clock high-activity mode) engages only
            # ~12us after the PE first goes busy; without this, the first
            # 2-3 chunks run at half clock (227ns vs 116ns per matmul).
            # Dummy matmuls on a memset tile pull the qualification window
            # forward to overlap the W/x0 load latency.
            wu_w = cpool.tile([128, 128], BF)
            nc.vector.memset(wu_w[:], 0.0)
            wu_r = opool.tile([128, 512], BF, name="wu_r", tag="osb")
            nc.gpsimd.memset(wu_r[:], 0.0)
            wu_ps = ampool.tile([128, 512], F32, name="wu", tag="am")
            for w in range(10):
                nc.tensor.matmul(
                    wu_ps[:], lhsT=wu_w[:], rhs=wu_r[:],
                    start=True, stop=True,
                )

            zts = [None] * NCHUNK    # per-chunk [32, 128] bf16 stationary
            osbs = [None] * NCHUNK   # per-chunk [128, 4096] bf16 out tiles

            def mix_tail(c, am, last=False):
                """After AM(c) PSUM completes: DVE mul (z read straight
                from PSUM), DVE reduce, ACT z copy, DVE 32x32 transposes,
                ACT bf16 downcast of the transposed stationary."""
                zc = mixpool.tile([128, 2 * RH], F32, tag="zc", name=f"zc{c}")
                # z copy (ACT): DVE cannot read both mul operands from PSUM
                nc.scalar.copy(zc[:, RH:2 * RH], am[:, MDIM:WCOLS])
                # P[p, i, j] = M[p, i, j] * z[p, j]
                p_sb = mixpool.tile([128, MDIM], BF, tag="p", name=f"pp{c}")
                nc.vector.tensor_mul(
                    p_sb[:].rearrange("p (i j) -> p i j", i=RH),
                    am[:, 0:MDIM].rearrange("p (i j) -> p i j", i=RH),
                    zc[:, RH:2 * RH].unsqueeze(1).broadcast_to([128, RH, RH]),
                )
                # z_mixed[p, i] = sum_j P[p, i, j]
                nc.vector.tensor_reduce(
                    zc[:, 0:RH], p_sb[:].rearrange("p (i j) -> p i j", i=RH),
                    axis=mybir.AxisListType.X, op=mybir.AluOpType.add,
                )
                # transpose [z_mixed | z] -> [32, 128] via 4 DVE 32x32 blocks
                zt_f = mixpool.tile([2 * RH, CHUNK], F32, tag="ztf", name=f"ztf{c}")
                for b in range(4):
                    nc.vector.transpose(
                        zt_f[:, b * 32:(b + 1) * 32],
                        zc[b * 32:(b + 1) * 32, :],
                    )
                # bf16 downcast: DVE normally (194ns right behind the
                # transposes); for the LAST chunk use Pool — the scheduler
                # otherwise hoists burst(6) evacuations above this copy on
                # DVE and the final burst slips into a clock-down window
                zt_sb = mixpool.tile([2 * RH, CHUNK], BF, tag="zt", name=f"zt{c}")
                if last:
                    nc.gpsimd.tensor_copy(zt_sb[:], zt_f[:])
                else:
                    nc.vector.tensor_copy(zt_sb[:], zt_f[:])
                zts[c] = zt_sb

            def b_burst(c, fast_tail=False):
                """8 B matmuls (K=32, N=512) into 3 double-bank PSUM
                tiles; 1024-wide evacuations ACT, ACT, DVE, DVE; stores
                of each 2048 half ride the sync FIFO behind the loads."""
                o_sb = opool.tile([128, OUT_D], BF, name=f"osb{c}", tag="osb")
                osbs[c] = o_sb
                tok = slice(c * CHUNK, (c + 1) * CHUNK)
                for pair in range(4):
                    bp = bpool.tile([128, 1024], F32, name=f"bp{c}_{pair}", tag="bp")
                    for half in range(2):
                        ob = 2 * pair + half
                        nc.tensor.matmul(
                            bp[:, half * 512:(half + 1) * 512],
                            lhsT=zts[c][:], rhs=btsb[:, ob * 512:(ob + 1) * 512],
                            start=True, stop=True,
                        )
                    osl = slice(pair * 1024, (pair + 1) * 1024)
                    if pair % 2 == 0:
                        nc.scalar.copy(o_sb[:, osl], bp[:])
                    else:
                        nc.vector.tensor_copy(o_sb[:, osl], bp[:])
                    if fast_tail:
                        # final chunk: store each 1024 as it lands, spread
                        # across the three HWDGE queues
                        q = [nc.sync, nc.scalar, nc.gpsimd, nc.scalar]
                        q[pair].dma_start(out[tok, osl], o_sb[:, osl])
                    elif pair == 3:
                        # one 1MB store per chunk: fewer end-of-run
                        # semaphore events (they serialize into the final
                        # barrier at ~115ns apiece).  Rotate the HWDGE ring
                        # per chunk: a single ring's descriptor dispatch
                        # only keeps the DMA engines ~50% fed during the
                        # store phase, while the other rings sit idle.
                        ring = [nc.sync, nc.scalar, nc.gpsimd][c % 3]
                        ring.dma_start(out[tok, :], o_sb[:])

            # software pipeline: per chunk the PE order is [AM(c), B(c-1)];
            # the mix chain of c is EMITTED before the burst of c-1 so the
            # DVE/ACT queues serve it first and zt(c) is ready well before
            # the PE reaches B(c)
            for c in range(NCHUNK):
                xs = xtiles[c]
                am = ampool.tile([128, WCOLS], F32, name=f"am{c}", tag="am")
                for k in range(KD):
                    nc.tensor.matmul(
                        am[:], lhsT=xs[:, k, :], rhs=wsb[:, k, :],
                        start=(k == 0), stop=(k == KD - 1),
                    )
                mix_tail(c, am, last=(c == NCHUNK - 1))
                if c >= 1:
                    b_burst(c - 1)
                if c == NCHUNK - 1:
                    # warm-keeper: the PE otherwise idles ~2us here waiting
                    # for zt(7), dropping out of high-activity mode and
                    # running the last burst at half clock.  Harmless
                    # matmuls into the retired am tile keep HAM up.
                    for w in range(16):
                        nc.tensor.matmul(
                            am[:], lhsT=xs[:, w % KD, :], rhs=wsb[:, w % KD, :],
                            start=True, stop=True,
                        )
            b_burst(NCHUNK - 1, fast_tail=True)

    nc.compile()
    return nc


_NC = None


def _get_nc():
    global _NC
    if _NC is None:
        _NC = build_nc()
    return _NC


def make_in_maps(x, A_w, B_w, M_w, M_b):
    x = np.asarray(x, dtype=np.float32)
    A_w = np.asarray(A_w, dtype=np.float32)
    B_w = np.asarray(B_w, dtype=np.float32)
    M_w = np.asarray(M_w, dtype=np.float32)
    M_b = np.asarray(M_b, dtype=np.float32)

    # fold scales: M_hat = x @ (8 M_w).T ; out = z_mixed @ (128 B_w).T + z @ (128 B_w MB).T
    W = np.concatenate([8.0 * M_w, A_w], axis=0)              # [272, 4096]
    wT_np = W.T.astype(NPBF)                                  # [4096, 272]
    # swizzle to [128, k*272 + m] so each SBUF partition line is contiguous
    wsw_np = np.ascontiguousarray(
        wT_np.reshape(KD, 128, WCOLS).transpose(1, 0, 2).reshape(128, KD * WCOLS)
    )
    MB = (8.0 * M_b).reshape(RH, RH)
    B1 = 128.0 * B_w                                          # [4096, 16]
    B2 = B1 @ MB                                              # [4096, 16]
    bT_np = np.ascontiguousarray(
        np.concatenate([B1.T, B2.T], axis=0)                  # [32, 4096]
    ).astype(NPBF)

    xf = x.reshape(N_TOK, D)
    in_maps = []
    for core in range(N_CORES):
        shard = xf[core * TPC:(core + 1) * TPC].astype(NPBF)  # [1024, 4096]
        # xsw[p, c*4096 + k*128 + t] = shard[c*128 + t, k*128 + p]
        xsw_np = np.ascontiguousarray(
            shard.reshape(NCHUNK, CHUNK, KD, 128)             # [c, t, k, p]
            .transpose(3, 0, 2, 1)                            # [p, c, k, t]
            .reshape(128, NCHUNK * KD * CHUNK)
        )
        in_maps.append({
            "xsw": xsw_np, "wsw": wsw_np, "bT": bT_np,
        })
    return in_maps


def assemble_out(results):
    outs = [np.asarray(results[i]["out"], dtype=np.float32) for i in range(N_CORES)]
    return np.concatenate(outs, axis=0).reshape(B, S, OUT_D)


def kernel(x, A_w, B_w, M_w, M_b):
    nc = _get_nc()
    in_maps = make_in_maps(x, A_w, B_w, M_w, M_b)
    res = run_bass_kernel_spmd(nc, in_maps, core_ids=list(range(N_CORES)))
    return assemble_out(res.results)
